# revision 13
# baseline (speedup 1.0000x reference)
"""DualAttention Trainium2 kernel: 8-way batch-parallel SPMD.

Per core (one batch element):
  spatial attn: St[k,q] = [K;-1]^T [Q;B_q] fp32r matmuls (K=65 contraction,
                per-query Cauchy-Schwarz shift B_q-70 baked into the 65th row),
                exp on ACT (psum->bf16), AV via bf16 matmuls with a
                ones-column producing the softmax denominator row.
  channel attn: scores [C, LC] fp32r accumulated over 8 WH-chunks, free-dim
                softmax (max-bias exp + accum denominator), PE-transpose,
                bf16 AV.
  stems: conv5x5 as 25 shifted 1D-stretch bf16 matmuls in a 36-wide padded
         layout (even/odd taps row-packed in partition halves), global LN via
         bn_stats + ones-matmul cross-partition sum, relu, 1x1 conv, bias.
  c-stem is emitted before spatial attention to fill the B_q/DMA ramp.
"""
import numpy as np

import concourse.bass as bass
import concourse.bass_isa as bass_isa
import concourse.mybir as mybir
import concourse.tile as tile
from concourse import bacc
from concourse.masks import make_identity
from concourse.bass_utils import run_bass_kernel_spmd

F32 = mybir.dt.float32
F32R = mybir.dt.float32r
BF16 = mybir.dt.bfloat16
AF = mybir.ActivationFunctionType
ALU = mybir.AluOpType
AX = mybir.AxisListType

B, L, C, W, NH, FS = 8, 8, 64, 32, 128, 5
WH = W * W            # 1024
LWH = L * WH          # 8192
LC = L * C            # 512
NKC = LWH // 128      # 64 key chunks
PADW = W + 4          # 36
XPAD_LEN = 1728
LN_EPS = 1e-5
BQ_OFF = 70.0

_CACHE = {}


def _build():
    nc = bacc.Bacc("TRN2", target_bir_lowering=False, debug=False, num_devices=8)

    q_cw = nc.declare_dram_parameter("q_cw", [C, WH], F32R, isOutput=False)
    kaug = nc.declare_dram_parameter("kaug", [65, LWH], F32R, isOutput=False)
    q_wc = nc.declare_dram_parameter("q_wc", [128, 8, C], F32R, isOutput=False)
    k_wc = nc.declare_dram_parameter("k_wc", [128, 8, LC], F32R, isOutput=False)
    v_aug = nc.declare_dram_parameter("v_aug", [128, NKC, 65], F32, isOutput=False)
    v_cw = nc.declare_dram_parameter("v_cw", [128, 4, WH], F32, isOutput=False)
    pw1, pw2, pb1, pb2, plnw, plnb = {}, {}, {}, {}, {}, {}
    for s in ("s", "c"):
        pw1[s] = nc.declare_dram_parameter(f"{s}_w1t", [128, 13 * 64], F32, isOutput=False)
        pw2[s] = nc.declare_dram_parameter(f"{s}_w2t", [C, NH], F32, isOutput=False)
        pb1[s] = nc.declare_dram_parameter(f"{s}_b1", [C, 1], F32, isOutput=False)
        pb2[s] = nc.declare_dram_parameter(f"{s}_b2", [NH, 1], F32, isOutput=False)
        plnw[s] = nc.declare_dram_parameter(f"{s}_lnw", [C, WH], F32, isOutput=False)
        plnb[s] = nc.declare_dram_parameter(f"{s}_lnb", [C, WH], F32, isOutput=False)
    s_out = nc.declare_dram_parameter("s_out", [NH, WH], F32, isOutput=True)
    c_out = nc.declare_dram_parameter("c_out", [NH, WH], F32, isOutput=True)
    outp = {"s": s_out, "c": c_out}

    with tile.TileContext(nc) as tc:
        with (
            tc.tile_pool(name="sb", bufs=1) as sb,
            tc.tile_pool(name="ps", bufs=1, space="PSUM") as ps,
        ):
            # ---- loads (split across queues; channel/B_q inputs first) ----
            kwc = sb.tile([128, 8, LC], F32R)
            for i in range(4):
                nc.sync.dma_start(out=kwc[:, 2 * i:2 * i + 2, :],
                                  in_=k_wc[:, 2 * i:2 * i + 2, :])
            qwc = sb.tile([128, 8, C], F32R)
            nc.sync.dma_start(out=qwc[:], in_=q_wc[:])
            qaug = sb.tile([65, WH], F32R)
            nc.sync.dma_start(out=qaug[0:C, :], in_=q_cw[:])
            kaug_sb = sb.tile([65, LWH], F32R)
            for i in range(4):
                nc.sync.dma_start(out=kaug_sb[:, 2048 * i:2048 * (i + 1)],
                                  in_=kaug[:, 2048 * i:2048 * (i + 1)])
            vcw = sb.tile([128, 4, WH], BF16)
            for i in range(2):
                nc.gpsimd.dma_start(out=vcw[:, 2 * i:2 * i + 2, :],
                                    in_=v_cw[:, 2 * i:2 * i + 2, :])
            vau = sb.tile([128, NKC, 65], BF16)
            for i in range(4):
                nc.gpsimd.dma_start(out=vau[:, 16 * i:16 * (i + 1), :],
                                    in_=v_aug[:, 16 * i:16 * (i + 1), :])
            w1t, w2t, b1t, b2t, lnwt, lnbt = {}, {}, {}, {}, {}, {}
            for s in ("c", "s"):
                w1t[s] = sb.tile([128, 13 * 64], BF16, tag=f"w1t{s}", name=f"w1t{s}")
                nc.gpsimd.dma_start(out=w1t[s][:], in_=pw1[s][:])
                w2t[s] = sb.tile([C, NH], BF16, tag=f"w2t{s}", name=f"w2t{s}")
                nc.gpsimd.dma_start(out=w2t[s][:], in_=pw2[s][:])
                b1t[s] = sb.tile([C, 1], F32, tag=f"b1{s}", name=f"b1{s}")
                nc.sync.dma_start(out=b1t[s][:], in_=pb1[s][:])
                b2t[s] = sb.tile([NH, 1], F32, tag=f"b2{s}", name=f"b2{s}")
                nc.sync.dma_start(out=b2t[s][:], in_=pb2[s][:])
                lnwt[s] = sb.tile([C, WH], F32, tag=f"lnw{s}", name=f"lnw{s}")
                nc.sync.dma_start(out=lnwt[s][:], in_=plnw[s][:])
                lnbt[s] = sb.tile([C, WH], F32, tag=f"lnb{s}", name=f"lnb{s}")
                nc.sync.dma_start(out=lnbt[s][:], in_=plnb[s][:])

            eps_t = sb.tile([C, 1], F32)
            nc.vector.memset(eps_t[:], LN_EPS)
            ones64 = sb.tile([C, C], F32)
            nc.vector.memset(ones64[:], 1.0)
            onec = sb.tile([C, 1], F32)
            nc.vector.memset(onec[:], 1.0)
            ones1 = sb.tile([1, C], F32)
            nc.vector.memset(ones1[:], 1.0)
            ident_bf = sb.tile([C, C], BF16)
            make_identity(nc, ident_bf[:])

            # ===== B_q = ||q|| * max_k ||k|| - BQ_OFF (spatial aug row) =====
            qsq = sb.tile([C, WH], F32, tag="y1")
            nc.vector.tensor_tensor(out=qsq[:], in0=qaug[0:C, :], in1=qaug[0:C, :],
                                    op=ALU.mult)
            normq = sb.tile([1, WH], F32)
            for h in range(2):
                nq_ps = ps.tile([1, 512], F32, tag="aux", name="nq_ps")
                nc.tensor.matmul(nq_ps[0:1, :], onec[:],
                                 qsq[:, h * 512:(h + 1) * 512], start=True, stop=True)
                nc.scalar.activation(out=normq[:, h * 512:(h + 1) * 512],
                                     in_=nq_ps[0:1, :], func=AF.Sqrt)
            ksq = sb.tile([128, 8, LC], F32, tag="big_scratch")
            nc.vector.tensor_tensor(out=ksq[:], in0=kwc[:], in1=kwc[:], op=ALU.mult)
            kn2 = sb.tile([128, 8, 8], F32)
            nc.vector.tensor_reduce(
                out=kn2[:], in_=ksq[:].rearrange("p a (l c) -> p a l c", c=C),
                axis=AX.X, op=ALU.add,
            )
            kn2m = sb.tile([128, 1], F32)
            nc.vector.tensor_reduce(out=kn2m[:], in_=kn2[:], axis=AX.XY, op=ALU.max)
            km2 = sb.tile([128, 1], F32)
            nc.gpsimd.partition_all_reduce(km2[:], kn2m[:], channels=128,
                                           reduce_op=bass_isa.ReduceOp.max)
            km = sb.tile([1, 1], F32)
            nc.scalar.activation(out=km[:], in_=km2[0:1, :], func=AF.Sqrt)
            bq = sb.tile([1, WH], F32)
            nc.vector.tensor_scalar_mul(bq[:], normq[:], km[:])
            nc.vector.tensor_scalar_sub(qaug[C:65, :], bq[:], BQ_OFF)

            # =================== channel attention ===================
            ch_sc = ps.tile([C, LC], F32, tag="av")
            for i in range(8):
                nc.tensor.matmul(ch_sc[:], qwc[:, i, :], kwc[:, i, :],
                                 start=(i == 0), stop=(i == 7))
            negmax = sb.tile([C, 1], F32)
            nc.vector.tensor_reduce(out=negmax[:], in_=ch_sc[:], axis=AX.X,
                                    op=ALU.max, negate=True)
            a_ch = sb.tile([C, LC], F32)
            denom_ch = sb.tile([C, 1], F32)
            nc.scalar.activation(out=a_ch[:], in_=ch_sc[:], func=AF.Exp,
                                 bias=negmax[:], scale=1.0, accum_out=denom_ch[:])
            rec_ch = sb.tile([C, 1], F32)
            nc.vector.reciprocal(out=rec_ch[:], in_=denom_ch[:])
            a_n = sb.tile([C, LC], BF16)
            nc.vector.tensor_scalar_mul(a_n[:], a_ch[:], rec_ch[:])
            ps_t = ps.tile([128, 4 * C], BF16, tag="aux")
            with nc.allow_low_precision(reason="bf16 PE transpose, no accumulation"):
                for j in range(4):
                    nc.tensor.transpose(ps_t[:, j * C:(j + 1) * C],
                                        a_n[:, j * 128:(j + 1) * 128], ident_bf[:])
            a_t = sb.tile([128, 4, C], BF16)
            nc.vector.tensor_copy(a_t[:], ps_t[:])
            ps_avch = ps.tile([C, WH], F32, tag="st", bufs=2)
            for j in range(4):
                for h in range(2):
                    nc.tensor.matmul(
                        ps_avch[:, h * 512:(h + 1) * 512],
                        a_t[:, j, :], vcw[:, j, h * 512:(h + 1) * 512],
                        start=(j == 0), stop=(j == 3),
                    )
            cx = sb.tile([C, WH], F32)
            nc.vector.tensor_tensor(out=cx[:], in0=ps_avch[:], in1=qaug[0:C, :],
                                    op=ALU.add)

            # =================== stems ===================
            def emit_stem(s, xin):
                xpad = sb.tile([128, XPAD_LEN], BF16, tag="xpad", name="xpad")
                nc.vector.memset(xpad[:], 0.0)
                xv = xpad[0:C, 0:1296].rearrange("p (h w) -> p h w", w=PADW)
                nc.vector.tensor_copy(
                    xv[:, 2:34, 2:34], xin[:].rearrange("p (h w) -> p h w", w=W)
                )
                nc.sync.dma_start(out=xpad[C:128, :], in_=xpad[0:C, :])
                psA = ps.tile([C, 1536], F32, tag="st", bufs=2, name="psA")
                psB = ps.tile([C, 1536], F32, tag="st", bufs=2, name="psB")
                for t in range(25):
                    off = (t // 5) * PADW + (t % 5)
                    lo = t % 2 == 0
                    wsl = (w1t[s][0:64, (t // 2) * 64:(t // 2 + 1) * 64] if lo
                           else w1t[s][64:128, (t // 2) * 64:(t // 2 + 1) * 64])
                    pdst = psA if lo else psB
                    for h in range(3):
                        xsl = (xpad[0:64, off + h * 512:off + (h + 1) * 512] if lo
                               else xpad[64:128, off + h * 512:off + (h + 1) * 512])
                        nc.tensor.matmul(pdst[:, h * 512:(h + 1) * 512], wsl, xsl,
                                         start=t in (0, 1), stop=t in (23, 24))
                pA_v = psA[:, 0:1152].rearrange("p (h w) -> p h w", w=PADW)[:, :, 0:W]
                pB_v = psB[:, 0:1152].rearrange("p (h w) -> p h w", w=PADW)[:, :, 0:W]
                psb_sb = sb.tile([C, WH], F32, tag="psb_sb", name="psb_sb")
                nc.scalar.copy(out=psb_sb[:].rearrange("p (h w) -> p h w", w=W),
                               in_=pB_v)
                y1 = sb.tile([C, WH], F32, tag="y1", name="y1")
                nc.vector.scalar_tensor_tensor(
                    out=y1[:].rearrange("p (h w) -> p h w", w=W),
                    in0=pA_v, scalar=b1t[s][:],
                    in1=psb_sb[:].rearrange("p (h w) -> p h w", w=W),
                    op0=ALU.add, op1=ALU.add,
                )
                stats = sb.tile([C, 2, 6], F32, tag="stats", name="stats")
                for h in range(2):
                    nc.vector.bn_stats(out=stats[:, h, :],
                                       in_=y1[:, h * 512:(h + 1) * 512])
                mv = sb.tile([C, 2], F32, tag="mv", name="mv")
                nc.vector.bn_aggr(out=mv[:], in_=stats[:])
                t2 = sb.tile([C, 2], F32, tag="t2", name="t2")
                nc.vector.tensor_copy(t2[:, 0:1], mv[:, 0:1])
                nc.vector.tensor_tensor(out=t2[:, 1:2], in0=mv[:, 0:1],
                                        in1=mv[:, 0:1], op=ALU.mult)
                nc.vector.tensor_tensor(out=t2[:, 1:2], in0=t2[:, 1:2],
                                        in1=mv[:, 1:2], op=ALU.add)
                ps_g = ps.tile([C, 2], F32, tag="aux", name="ps_g")
                nc.tensor.matmul(ps_g[:], ones64[:], t2[:], start=True, stop=True)
                g = sb.tile([C, 2], F32, tag="g", name="g")
                nc.scalar.mul(g[:], ps_g[:], 1.0 / 64.0)
                mu2 = sb.tile([C, 1], F32, tag="mu2", name="mu2")
                nc.vector.tensor_tensor(out=mu2[:], in0=g[:, 0:1], in1=g[:, 0:1],
                                        op=ALU.mult)
                varg = sb.tile([C, 1], F32, tag="varg", name="varg")
                nc.vector.tensor_tensor(out=varg[:], in0=g[:, 1:2], in1=mu2[:],
                                        op=ALU.subtract)
                std = sb.tile([C, 1], F32, tag="std", name="std")
                nc.scalar.activation(out=std[:], in_=varg[:], func=AF.Sqrt,
                                     bias=eps_t[:], scale=1.0)
                istd = sb.tile([C, 1], F32, tag="istd", name="istd")
                nc.vector.reciprocal(out=istd[:], in_=std[:])
                lnw_s = sb.tile([C, WH], F32, tag="lnw_s", name="lnw_s")
                nc.vector.tensor_scalar_mul(lnw_s[:], lnwt[s][:], istd[:])
                t3 = sb.tile([C, WH], F32, tag="t3", name="t3")
                nc.vector.scalar_tensor_tensor(out=t3[:], in0=y1[:],
                                               scalar=g[:, 0:1], in1=lnw_s[:],
                                               op0=ALU.subtract, op1=ALU.mult)
                t4 = sb.tile([C, WH], F32, tag="t4", name="t4")
                nc.vector.tensor_tensor(out=t4[:], in0=t3[:], in1=lnbt[s][:],
                                        op=ALU.add)
                relu_y = sb.tile([C, WH], BF16, tag="relu_y", name="relu_y")
                nc.vector.tensor_scalar_max(relu_y[:], t4[:], 0.0)
                ps2 = ps.tile([NH, WH], F32, tag="st", bufs=2, name="ps2")
                for h in range(2):
                    nc.tensor.matmul(ps2[:, h * 512:(h + 1) * 512], w2t[s][:],
                                     relu_y[:, h * 512:(h + 1) * 512],
                                     start=True, stop=True)
                out_t = sb.tile([NH, WH], F32, tag="out_t", name="out_t")
                nc.vector.tensor_scalar_add(out_t[:], ps2[:], b2t[s][:])
                nc.sync.dma_start(out=outp[s][:], in_=out_t[:])

            emit_stem("c", cx)

            # =================== spatial attention ===================
            sx = sb.tile([C, WH], F32)
            for qq in range(2):
                av_ps = ps.tile([65, 512], F32, tag="av")
                nav = 0
                groups = [(3 * j, min(3, NKC - 3 * j)) for j in range((NKC + 2) // 3)]
                for (k0, glen) in groups:
                    st_ps = ps.tile([128, 3 * 512], F32, tag="st", bufs=2)
                    for bi in range(glen):
                        kc = k0 + bi
                        nc.tensor.matmul(
                            st_ps[:, bi * 512:(bi + 1) * 512],
                            kaug_sb[:, kc * 128:(kc + 1) * 128],
                            qaug[:, qq * 512:(qq + 1) * 512],
                            start=True, stop=True,
                        )
                    etj = sb.tile([128, 3, 512], BF16, tag="et", bufs=6, name="etj")
                    nc.scalar.activation(out=etj[:, 0:glen, :],
                                         in_=st_ps[:, 0:glen * 512], func=AF.Exp)
                    for bi in range(glen):
                        kc = k0 + bi
                        nc.tensor.matmul(av_ps[:], vau[:, kc, :], etj[:, bi, :],
                                         start=(nav == 0), stop=(nav == NKC - 1))
                        nav += 1
                rec_sp = sb.tile([1, 512], F32, tag="rec_sp", name="rec_sp")
                nc.vector.reciprocal(out=rec_sp[:], in_=av_ps[64:65, :])
                bc_ps = ps.tile([C, 512], F32, tag="aux", name="bc_ps")
                nc.tensor.matmul(bc_ps[:], ones1[:], rec_sp[:], start=True, stop=True)
                rb = sb.tile([C, 512], F32, tag="rb", name="rb")
                nc.vector.tensor_copy(rb[:], bc_ps[:])
                tmp_sp = sb.tile([C, 512], F32, tag="tmp_sp", name="tmp_sp")
                nc.vector.tensor_tensor(out=tmp_sp[:], in0=av_ps[0:C, :], in1=rb[:],
                                        op=ALU.mult)
                nc.vector.tensor_tensor(
                    out=sx[:, qq * 512:(qq + 1) * 512], in0=tmp_sp[:],
                    in1=qaug[0:C, qq * 512:(qq + 1) * 512], op=ALU.add,
                )

            emit_stem("s", sx)

    nc.compile()
    return nc


def _prep(in_query, in_keys, in_values,
          s_w1, s_b1, s_lnw, s_lnb, s_w2, s_b2,
          c_w1, c_b1, c_lnw, c_lnb, c_w2, c_b2):
    f32 = np.float32
    shared = {}
    for s, w1, b1, lnw, lnb, w2, b2 in (
        ("s", s_w1, s_b1, s_lnw, s_lnb, s_w2, s_b2),
        ("c", c_w1, c_b1, c_lnw, c_lnb, c_w2, c_b2),
    ):
        taps = w1.reshape(C, C, 25)
        lowc = np.zeros((64, 13 * 64), f32)
        hic = np.zeros((64, 13 * 64), f32)
        for t in range(25):
            wt = np.ascontiguousarray(taps[:, :, t].T)
            j = t // 2
            if t % 2 == 0:
                lowc[:, j * 64:(j + 1) * 64] = wt
            else:
                hic[:, j * 64:(j + 1) * 64] = wt
        shared[f"{s}_w1t"] = np.concatenate([lowc, hic], 0)
        shared[f"{s}_w2t"] = np.ascontiguousarray(w2[:, :, 0, 0].T, dtype=f32)
        shared[f"{s}_b1"] = np.ascontiguousarray(b1.reshape(C, 1), dtype=f32)
        shared[f"{s}_b2"] = np.ascontiguousarray(b2.reshape(NH, 1), dtype=f32)
        shared[f"{s}_lnw"] = np.ascontiguousarray(lnw.reshape(C, WH), dtype=f32)
        shared[f"{s}_lnb"] = np.ascontiguousarray(lnb.reshape(C, WH), dtype=f32)
    maps = []
    for b in range(B):
        m = dict(shared)
        m["q_cw"] = np.ascontiguousarray(in_query[b].reshape(C, WH), dtype=f32)
        kk = in_keys[b].reshape(L, C, WH)
        m["kaug"] = np.concatenate(
            [kk.transpose(1, 0, 2).reshape(C, LWH), -np.ones((1, LWH), f32)], 0
        ).astype(f32)
        qwcf = in_query[b].reshape(C, WH).T.reshape(8, 128, C)
        m["q_wc"] = np.ascontiguousarray(qwcf.transpose(1, 0, 2), dtype=f32)
        kwcf = in_keys[b].transpose(2, 3, 0, 1).reshape(8, 128, LC)
        m["k_wc"] = np.ascontiguousarray(kwcf.transpose(1, 0, 2), dtype=f32)
        va = in_values[b].transpose(0, 2, 3, 1).reshape(LWH, C)
        va = np.concatenate([va, np.ones((LWH, 1), f32)], 1)
        m["v_aug"] = np.ascontiguousarray(
            va.reshape(NKC, 128, 65).transpose(1, 0, 2), dtype=f32)
        vcwf = in_values[b].reshape(4, 128, WH)
        m["v_cw"] = np.ascontiguousarray(vcwf.transpose(1, 0, 2), dtype=f32)
        maps.append(m)
    return maps


def kernel(**inputs):
    inputs = {k: np.asarray(v, dtype=np.float32) for k, v in inputs.items()}
    if "nc" not in _CACHE:
        _CACHE["nc"] = _build()
    nc = _CACHE["nc"]
    maps = _prep(**inputs)
    res = run_bass_kernel_spmd(nc, maps, core_ids=list(range(8)))
    s = np.stack([res.results[b]["s_out"] for b in range(B)]).reshape(B, NH, W, W)
    c = np.stack([res.results[b]["c_out"] for b in range(B)]).reshape(B, NH, W, W)
    return (s.astype(np.float32), c.astype(np.float32))
